# revision 8
# baseline (speedup 1.0000x reference)
"""BasesDecomposition GNN message passing on 8 Trainium2 NeuronCores.

Math (reference):
    seg  = edge_type * N + target
    h    = segment_sum(x[source] * ew, seg)        # (R, N, D)
    out  = einsum('rb,bio,rni->no', bw, bases, h)  # (N, D)

Restructured: fold the relation->basis projection and edge weight into a
per-edge selector row  s4[e, b*M + m] = bw[edge_type_e, b] * ew_e * [tgt==m]
so the accumulator shrinks to (B, M, D) per 32-node tile:
    pg[i, (b,m)] = sum_e xg[e, i] * s4[e, (b,m)]      (PE, edge tiles of 128)
    out[m, o]    = sum_b pg[:, (b,m)]^T @ bases[b]    (PE)

Sharding: nodes by target range across 8 cores (no collectives). Edges are
sorted by (supergroup of 8 tiles, src-half, tile) and packed into a uniform
slot grid (T_LO/T_HI 128-slot chunks per tile-half) so the SPMD program is
static. s4 is built entirely on the host (no DVE work per edge tile); x is
cast to bf16 on the host into two tables split at node 25000 so dma_gather
indices fit int16. Gather calls are 1024 idx each, spread round-robin over
4 SWDGE queues (queue q runs on Q7 core pair q -> ~4x descriptor-gen
throughput vs one queue).
"""

import numpy as np

import concourse.bass as bass
import concourse.mybir as mybir
import concourse.tile as tile
from concourse import bacc
from concourse.bass_utils import run_bass_kernel_spmd

NCORES = 8
P = 128          # edges per chunk (matmul contraction dim)
M = 32           # nodes per tile (selector block width; B*M = 128 = J)
B = 4            # num bases
G = 8            # tiles per supergroup (psum tiles live concurrently)
SPLIT = 25000    # x row split so gather indices fit int16
CALL = 1024      # idxs per dma_gather call (hard ucode cap)
NQ = 4           # SWDGE queues used round-robin

TRACE = False
LAST_PROFILE = None

_PROG_CACHE = {}


def _build_program(N, D, NPC, NT_PAD, SG, T_LO, T_HI):
    fp = mybir.dt.float32
    bf = mybir.dt.bfloat16
    i16 = mybir.dt.int16
    T = T_LO + T_HI
    NLO = SPLIT
    NHI = N - SPLIT
    slots_sg = G * T * P              # slots per supergroup
    calls_lo = G * T_LO * P // CALL   # gather calls per supergroup (lo)
    calls_hi = G * T_HI * P // CALL
    assert G * T_LO * P % CALL == 0 and G * T_HI * P % CALL == 0

    nc = bacc.Bacc("TRN2", target_bir_lowering=False, debug=False,
                   num_devices=NCORES, num_swdge_queues=NQ)
    xlo_d = nc.dram_tensor("xlo", [NLO, D], bf, kind="ExternalInput").ap()
    xhi_d = nc.dram_tensor("xhi", [NHI, D], bf, kind="ExternalInput").ap()
    bases_d = nc.dram_tensor("bases16", [P, B * D], bf, kind="ExternalInput").ap()
    idx_d = nc.dram_tensor("idx16", [SG, P, slots_sg // P * 8], i16,
                           kind="ExternalInput").ap()
    s4_d = nc.dram_tensor("s4", [SG, P, G * T * B * M], bf,
                          kind="ExternalInput").ap()
    out_d = nc.dram_tensor("out", [NPC, D], fp, kind="ExternalOutput").ap()

    with tile.TileContext(nc) as tc:
        with (
            tc.tile_pool(name="const", bufs=1) as constp,
            tc.tile_pool(name="meta", bufs=2) as metap,
            tc.tile_pool(name="s4p", bufs=2) as s4p,
            tc.tile_pool(name="xg", bufs=2) as xgp,
            tc.tile_pool(name="gsb", bufs=2) as gsbp,
            tc.tile_pool(name="osb", bufs=3) as osbp,
            tc.tile_pool(name="psg", bufs=4, space="PSUM") as psgp,
            tc.tile_pool(name="pso", bufs=2, space="PSUM") as psop,
        ):
            bases_sb = constp.tile([P, B * D], bf)
            nc.sync.dma_start(out=bases_sb[:], in_=bases_d[:])

            call_no = 0
            for sg in range(SG):
                idxt = metap.tile([P, slots_sg // P * 8], i16, tag="idx")
                nc.sync.dma_start(out=idxt[:], in_=idx_d[sg])
                s4t = s4p.tile([P, G * T * B * M], bf, tag="s4")
                nc.sync.dma_start(out=s4t[:], in_=s4_d[sg])

                # gathers: lo block then hi block, 1024 idx per call
                xg_lo = xgp.tile([P, G * T_LO * D], bf, tag="xglo")
                xg_hi = xgp.tile([P, G * T_HI * D], bf, tag="xghi")
                for k in range(calls_lo):
                    nc.gpsimd.dma_gather(
                        out_ap=xg_lo[:, k * 8 * D:(k * 8 + 8) * D].rearrange(
                            "p (t f) -> p t f", f=D),
                        in_ap=xlo_d[:],
                        idxs_ap=idxt[:, k * 64:(k + 1) * 64],
                        num_idxs=CALL,
                        num_idxs_reg=CALL,
                        elem_size=D,
                        queue_num=call_no % NQ,
                    )
                    call_no += 1
                hi0 = calls_lo * 64
                for k in range(calls_hi):
                    nc.gpsimd.dma_gather(
                        out_ap=xg_hi[:, k * 8 * D:(k * 8 + 8) * D].rearrange(
                            "p (t f) -> p t f", f=D),
                        in_ap=xhi_d[:],
                        idxs_ap=idxt[:, hi0 + k * 64:hi0 + (k + 1) * 64],
                        num_idxs=CALL,
                        num_idxs_reg=CALL,
                        elem_size=D,
                        queue_num=call_no % NQ,
                    )
                    call_no += 1

                # s4 chunk layout: lo chunks (G*T_LO) then hi chunks (G*T_HI),
                # in slot order; chunk c of tile q:
                #   lo: c = q*T_LO + t, hi: c = G*T_LO + q*T_HI + t
                pg = None
                for q in range(G):
                    qq = q % 4
                    if qq == 0:
                        pg = psgp.tile([P, 4 * B * M], fp)
                    for t in range(T_LO):
                        c = q * T_LO + t
                        nc.tensor.matmul(
                            out=pg[:, qq * B * M:(qq + 1) * B * M],
                            lhsT=xg_lo[:, c * D:(c + 1) * D],
                            rhs=s4t[:, c * B * M:(c + 1) * B * M],
                            start=(t == 0),
                            stop=False,
                        )
                    for t in range(T_HI):
                        c = q * T_HI + t
                        nc.tensor.matmul(
                            out=pg[:, qq * B * M:(qq + 1) * B * M],
                            lhsT=xg_hi[:, c * D:(c + 1) * D],
                            rhs=s4t[:, (G * T_LO + c) * B * M:
                                    (G * T_LO + c + 1) * B * M],
                            start=False,
                            stop=(t == T_HI - 1),
                        )
                    if qq == 3:
                        quad = q // 4
                        # transpose-copy psum [p,(q,b,m)] -> sbuf [p,(b,q,m)]
                        gsb = gsbp.tile([P, 4 * B * M], bf, tag="gsb")
                        nc.vector.tensor_copy(
                            out=gsb[:].rearrange("p (b q m) -> p b q m",
                                                 b=B, q=4),
                            in_=pg[:].rearrange("p (q b m) -> p b q m",
                                                q=4, b=B),
                        )
                        po = psop.tile([P, D], fp)
                        for b in range(B):
                            nc.tensor.matmul(
                                out=po[:],
                                lhsT=gsb[:, b * 4 * M:(b + 1) * 4 * M],
                                rhs=bases_sb[:, b * D:(b + 1) * D],
                                start=(b == 0),
                                stop=(b == B - 1),
                            )
                        m_lo = (sg * G + quad * 4) * M
                        m_sz = min(4 * M, max(0, NPC - m_lo))
                        if m_sz > 0:
                            osb = osbp.tile([P, D], fp)
                            nc.vector.tensor_copy(out=osb[:m_sz, :],
                                                  in_=po[:m_sz, :])
                            nc.sync.dma_start(out=out_d[m_lo:m_lo + m_sz, :],
                                              in_=osb[:m_sz, :])
    nc.compile()
    return nc


def _wrap16(a):
    """Pack flat index array (n,) into dma_gather layout (128, n/16):
    index j lives at [j % 16, j // 16]; rows replicated to 128."""
    n = a.shape[0]
    w = a.reshape(n // 16, 16).T  # (16, n/16)
    return np.tile(w, (8, 1))


def kernel(x, source, target, edge_type, edge_weights, base_weights, bases):
    global LAST_PROFILE
    import ml_dtypes

    x = np.ascontiguousarray(np.asarray(x), dtype=np.float32)
    src = np.asarray(source).astype(np.int64)
    tgt = np.asarray(target).astype(np.int64)
    et = np.asarray(edge_type).astype(np.int64)
    ew = np.ascontiguousarray(np.asarray(edge_weights), dtype=np.float32)
    bw = np.ascontiguousarray(np.asarray(base_weights), dtype=np.float32)
    bs = np.ascontiguousarray(np.asarray(bases), dtype=np.float32)

    N, D = x.shape
    R, Bv = bw.shape
    E = src.shape[0]
    NPC = N // NCORES
    NT = (NPC + M - 1) // M           # real node tiles per core
    NT_PAD = (NT + G - 1) // G * G    # padded to supergroup multiple
    SG = NT_PAD // G

    # ---- host: bf16 tables (even split so idx fits int16) ----
    xlo = x[:SPLIT].astype(ml_dtypes.bfloat16)
    xhi = x[SPLIT:].astype(ml_dtypes.bfloat16)
    bases16 = np.zeros((P, B * D), dtype=ml_dtypes.bfloat16)
    for b in range(B):
        bases16[:, b * D:(b + 1) * D] = bs[b].astype(ml_dtypes.bfloat16)

    # ---- host: edge partitioning ----
    core = tgt // NPC
    local = tgt - core * NPC
    ntile = local // M                # node tile within core
    m_loc = local - ntile * M         # node within tile
    sg_id = ntile // G
    half = (src >= SPLIT).astype(np.int64)

    # group key: (core, supergroup, half, tile)
    gid = ((core * SG + sg_id) * 2 + half) * NT_PAD + ntile
    # T per (tile, half): global max
    cnt = np.bincount((core * NT + ntile) * 2 + half,
                      minlength=NCORES * NT * 2).reshape(-1, 2)
    T_LO = int(np.ceil(cnt[:, 0].max() / P))
    T_HI = int(np.ceil(cnt[:, 1].max() / P))
    # pad so per-supergroup call blocks are CALL-aligned
    while G * T_LO * P % CALL:
        T_LO += 1
    while G * T_HI * P % CALL:
        T_HI += 1
    T = T_LO + T_HI

    order = np.argsort(gid, kind="stable")
    src_s = src[order]
    et_s = et[order]
    ew_s = ew[order]
    half_s = half[order]
    core_s = core[order]
    sg_s = sg_id[order]
    ntile_s = ntile[order]
    m_s = m_loc[order]

    # slot of each edge in the flat per-core grid:
    # core-base + sg*slots_sg + half-block + (tile within sg)*Tx*P + pos
    gid_s = gid[order]
    starts = np.zeros(gid.max() + 2, dtype=np.int64)
    cnts = np.bincount(gid_s, minlength=gid.max() + 1)
    np.cumsum(cnts, out=starts[1:])
    pos = np.arange(E, dtype=np.int64) - starts[gid_s]

    slots_sg = G * T * P
    tile_in_sg = ntile_s - sg_s * G
    Tx = np.where(half_s == 0, T_LO, T_HI)
    block0 = np.where(half_s == 0, 0, G * T_LO * P)
    slot_in_sg = block0 + tile_in_sg * Tx * P + pos
    slot = (core_s * SG + sg_s) * slots_sg + slot_in_sg

    nslots = NCORES * SG * slots_sg
    idx_flat = np.zeros(nslots, dtype=np.int16)
    idx_flat[slot] = (src_s - half_s * SPLIT).astype(np.int16)

    # wrapped idx per (core, sg): lo block then hi block
    idx_rs = idx_flat.reshape(NCORES, SG, slots_sg)
    nlo = G * T_LO * P
    idx16 = np.empty((NCORES, SG, P, slots_sg // P * 8), dtype=np.int16)
    for c in range(NCORES):
        for s in range(SG):
            idx16[c, s, :, :nlo // 16] = _wrap16(idx_rs[c, s, :nlo])
            idx16[c, s, :, nlo // 16:] = _wrap16(idx_rs[c, s, nlo:])

    # ---- host: s4 selector, [core, sg, chunk, p, col] col = b*M + m ----
    bw16 = (bw.astype(np.float32))  # keep f32 for product, cast at store
    chunk_in_sg = slot_in_sg // P
    p_in_chunk = slot_in_sg % P
    s4 = np.zeros((NCORES, SG, G * T, P, B * M), dtype=ml_dtypes.bfloat16)
    cw = bw16[et_s] * ew_s[:, None]          # (E, B)
    for b in range(B):
        s4[core_s, sg_s, chunk_in_sg, p_in_chunk, b * M + m_s] = \
            cw[:, b].astype(ml_dtypes.bfloat16)
    s4 = s4.reshape(NCORES, SG, G * T, P, B * M).transpose(0, 1, 3, 2, 4)
    s4 = np.ascontiguousarray(s4).reshape(NCORES, SG, P, G * T * B * M)

    key = (N, D, NPC, NT_PAD, SG, T_LO, T_HI)
    if key not in _PROG_CACHE:
        _PROG_CACHE[key] = _build_program(*key)
    nc = _PROG_CACHE[key]

    in_maps = [
        dict(
            xlo=xlo,
            xhi=xhi,
            bases16=bases16,
            idx16=idx16[c],
            s4=s4[c],
        )
        for c in range(NCORES)
    ]
    res = run_bass_kernel_spmd(nc, in_maps, list(range(NCORES)), trace=TRACE)
    LAST_PROFILE = res
    out = np.concatenate([res.results[c]["out"] for c in range(NCORES)], axis=0)
    return out


# revision 16
# speedup vs baseline: 1.0883x; 1.0883x over previous
"""BasesDecomposition GNN message passing on 8 Trainium2 NeuronCores.

Math (reference):
    seg  = edge_type * N + target
    h    = segment_sum(x[source] * ew, seg)        # (R, N, D)
    out  = einsum('rb,bio,rni->no', bw, bases, h)  # (N, D)

Restructured: fold the relation->basis projection and edge weight into a
per-edge selector row  s4[e, b*M + m] = bw[edge_type_e, b] * ew_e * [tgt==m]
so the accumulator shrinks to (B, M, D) per 32-node tile:
    pg[i, (b,m)] = sum_e xg[e, i] * s4[e, (b,m)]      (PE, edge tiles of 128)
    out[m, o]    = sum_b pg[:, (b,m)]^T @ bases[b]    (PE)

Sharding: nodes by target range across 8 cores (no collectives). Edges are
sorted by (supergroup of 8 tiles, src-half, tile) and packed into a uniform
slot grid (T_LO/T_HI 128-slot chunks per tile-half) so the SPMD program is
static. s4 is built entirely on the host (no DVE work per edge tile); x is
cast to bf16 on the host into two tables split at node 25000 so dma_gather
indices fit int16. Gather calls are 1024 idx each, spread round-robin over
4 SWDGE queues (queue q runs on Q7 core pair q -> ~4x descriptor-gen
throughput vs one queue).
"""

import numpy as np

import concourse.bass as bass
import concourse.mybir as mybir
import concourse.tile as tile
from concourse import bacc
from concourse.bass_utils import run_bass_kernel_spmd

NCORES = 8
P = 128          # edges per chunk (matmul contraction dim)
M = 32           # nodes per tile (selector block width; B*M = 128 = J)
B = 4            # num bases
G = 8            # tiles per supergroup (psum tiles live concurrently)
SPLIT = 22000    # x row split so gather indices fit int16 (tuned: T_LO=4, T_HI=5)
CALL = 1024      # idxs per dma_gather call (hard ucode cap)
NQ = 4           # SWDGE queues used round-robin
NT_PAD = 200     # node tiles per core (NCORES*NT_PAD*M = 51200 >= N slots)

TRACE = False
LAST_PROFILE = None

_PROG_CACHE = {}


def _build_program(N, D, NOUT, SG, T_LO, T_HI):
    fp = mybir.dt.float32
    bf = mybir.dt.bfloat16
    i16 = mybir.dt.int16
    T = T_LO + T_HI
    NLO = SPLIT
    NHI = N - SPLIT
    slots_sg = G * T * P              # slots per supergroup
    calls_lo = G * T_LO * P // CALL   # gather calls per supergroup (lo)
    calls_hi = G * T_HI * P // CALL
    assert G * T_LO * P % CALL == 0 and G * T_HI * P % CALL == 0

    nc = bacc.Bacc("TRN2", target_bir_lowering=False, debug=False,
                   num_devices=NCORES, num_swdge_queues=NQ)
    xlo_d = nc.dram_tensor("xlo", [NLO, D], bf, kind="ExternalInput").ap()
    xhi_d = nc.dram_tensor("xhi", [NHI, D], bf, kind="ExternalInput").ap()
    bases_d = nc.dram_tensor("bases16", [P, B * D], bf, kind="ExternalInput").ap()
    idx_d = nc.dram_tensor("idx16", [SG, P, slots_sg // P * 8], i16,
                           kind="ExternalInput").ap()
    s4_d = nc.dram_tensor("s4", [SG, P, G * T * B * M], bf,
                          kind="ExternalInput").ap()
    out_d = nc.dram_tensor("out", [NOUT, D], fp, kind="ExternalOutput").ap()

    with tile.TileContext(nc) as tc:
        with (
            tc.tile_pool(name="const", bufs=1) as constp,
            tc.tile_pool(name="meta", bufs=3) as metap,
            tc.tile_pool(name="s4p", bufs=3) as s4p,
            tc.tile_pool(name="xg", bufs=3) as xgp,
            tc.tile_pool(name="gsb", bufs=2) as gsbp,
            tc.tile_pool(name="osb", bufs=3) as osbp,
            tc.tile_pool(name="psg", bufs=4, space="PSUM") as psgp,
            tc.tile_pool(name="pso", bufs=2, space="PSUM") as psop,
        ):
            bases_sb = constp.tile([P, B * D], bf)
            nc.sync.dma_start(out=bases_sb[:], in_=bases_d[:])

            call_no = 0
            for sg in range(SG):
                idxt = metap.tile([P, slots_sg // P * 8], i16, tag="idx")
                nc.sync.dma_start(out=idxt[:], in_=idx_d[sg])
                s4t = s4p.tile([P, G * T * B * M], bf, tag="s4")
                nc.sync.dma_start(out=s4t[:], in_=s4_d[sg])

                # gathers: lo block then hi block, 1024 idx per call
                xg_lo = xgp.tile([P, G * T_LO * D], bf, tag="xglo")
                xg_hi = xgp.tile([P, G * T_HI * D], bf, tag="xghi")
                for k in range(calls_lo):
                    nc.gpsimd.dma_gather(
                        out_ap=xg_lo[:, k * 8 * D:(k * 8 + 8) * D].rearrange(
                            "p (t f) -> p t f", f=D),
                        in_ap=xlo_d[:],
                        idxs_ap=idxt[:, k * 64:(k + 1) * 64],
                        num_idxs=CALL,
                        num_idxs_reg=CALL,
                        elem_size=D,
                        queue_num=call_no % NQ,
                    )
                    call_no += 1
                hi0 = calls_lo * 64
                for k in range(calls_hi):
                    nc.gpsimd.dma_gather(
                        out_ap=xg_hi[:, k * 8 * D:(k * 8 + 8) * D].rearrange(
                            "p (t f) -> p t f", f=D),
                        in_ap=xhi_d[:],
                        idxs_ap=idxt[:, hi0 + k * 64:hi0 + (k + 1) * 64],
                        num_idxs=CALL,
                        num_idxs_reg=CALL,
                        elem_size=D,
                        queue_num=call_no % NQ,
                    )
                    call_no += 1

                # s4 chunk layout: lo chunks (G*T_LO) then hi chunks (G*T_HI),
                # in slot order; chunk c of tile q:
                #   lo: c = q*T_LO + t, hi: c = G*T_LO + q*T_HI + t
                pg = None
                for q in range(G):
                    qq = q % 4
                    if qq == 0:
                        pg = psgp.tile([P, 4 * B * M], fp)
                    for t in range(T_LO):
                        c = q * T_LO + t
                        nc.tensor.matmul(
                            out=pg[:, qq * B * M:(qq + 1) * B * M],
                            lhsT=xg_lo[:, c * D:(c + 1) * D],
                            rhs=s4t[:, c * B * M:(c + 1) * B * M],
                            start=(t == 0),
                            stop=False,
                        )
                    for t in range(T_HI):
                        c = q * T_HI + t
                        nc.tensor.matmul(
                            out=pg[:, qq * B * M:(qq + 1) * B * M],
                            lhsT=xg_hi[:, c * D:(c + 1) * D],
                            rhs=s4t[:, (G * T_LO + c) * B * M:
                                    (G * T_LO + c + 1) * B * M],
                            start=False,
                            stop=(t == T_HI - 1),
                        )
                    if qq == 3:
                        quad = q // 4
                        # transpose-copy psum [p,(q,b,m)] -> sbuf [p,(b,q,m)]
                        gsb = gsbp.tile([P, 4 * B * M], bf, tag="gsb")
                        nc.vector.tensor_copy(
                            out=gsb[:].rearrange("p (b q m) -> p b q m",
                                                 b=B, q=4),
                            in_=pg[:].rearrange("p (q b m) -> p b q m",
                                                q=4, b=B),
                        )
                        po = psop.tile([P, D], fp)
                        for b in range(B):
                            nc.tensor.matmul(
                                out=po[:],
                                lhsT=gsb[:, b * 4 * M:(b + 1) * 4 * M],
                                rhs=bases_sb[:, b * D:(b + 1) * D],
                                start=(b == 0),
                                stop=(b == B - 1),
                            )
                        m_lo = (sg * G + quad * 4) * M
                        osb = osbp.tile([P, D], fp)
                        nc.vector.tensor_copy(out=osb[:], in_=po[:])
                        nc.sync.dma_start(out=out_d[m_lo:m_lo + 4 * M, :],
                                          in_=osb[:])
    nc.compile()
    return nc


def _wrap16(a):
    """Pack flat index array (n,) into dma_gather layout (128, n/16):
    index j lives at [j % 16, j // 16]; rows replicated to 128."""
    n = a.shape[0]
    w = a.reshape(n // 16, 16).T  # (16, n/16)
    return np.tile(w, (8, 1))


def kernel(x, source, target, edge_type, edge_weights, base_weights, bases):
    global LAST_PROFILE
    import ml_dtypes

    x = np.ascontiguousarray(np.asarray(x), dtype=np.float32)
    src = np.asarray(source).astype(np.int64)
    tgt = np.asarray(target).astype(np.int64)
    et = np.asarray(edge_type).astype(np.int64)
    ew = np.ascontiguousarray(np.asarray(edge_weights), dtype=np.float32)
    bw = np.ascontiguousarray(np.asarray(base_weights), dtype=np.float32)
    bs = np.ascontiguousarray(np.asarray(bases), dtype=np.float32)

    N, D = x.shape
    R, Bv = bw.shape
    E = src.shape[0]
    SG = NT_PAD // G
    NOUT = NT_PAD * M                 # device output rows per core

    # ---- host: bf16 tables (split so gather idx fits int16) ----
    xlo = x[:SPLIT].astype(ml_dtypes.bfloat16)
    xhi = x[SPLIT:].astype(ml_dtypes.bfloat16)
    bases16 = np.zeros((P, B * D), dtype=ml_dtypes.bfloat16)
    for b in range(B):
        bases16[:, b * D:(b + 1) * D] = bs[b].astype(ml_dtypes.bfloat16)

    # ---- host: balanced node -> (core, tile, m) assignment ----
    # Nodes can be placed in any output slot (we un-permute on the host), so
    # rank-match nodes onto tiles to equalize per-(tile, half) edge counts,
    # minimizing the uniform T padding of the static SPMD program.
    half_e = (src >= SPLIT).astype(np.int64)
    lo_deg = np.bincount(tgt[half_e == 0], minlength=N).astype(np.int64)
    hi_deg = np.bincount(tgt[half_e == 1], minlength=N).astype(np.int64)
    TILES = NCORES * NT_PAD
    node_order = np.argsort(-(lo_deg + hi_deg), kind="stable")
    tile_lo = np.zeros(TILES, dtype=np.int64)
    tile_hi = np.zeros(TILES, dtype=np.int64)
    tile_cnt = np.zeros(TILES, dtype=np.int64)
    gtile_of = np.empty(N, dtype=np.int64)
    m_of = np.empty(N, dtype=np.int64)
    for r0 in range(0, N, TILES):
        batch = node_order[r0:r0 + TILES]
        load = tile_lo * 5 + tile_hi * 4          # ~ lo/T_LO + hi/T_HI
        rank = np.argsort(load, kind="stable")
        chosen = rank[:len(batch)]
        gtile_of[batch] = chosen
        m_of[batch] = tile_cnt[chosen]
        tile_cnt[chosen] += 1
        tile_lo[chosen] += lo_deg[batch]
        tile_hi[chosen] += hi_deg[batch]
    assert tile_cnt.max() <= M

    T_LO = int(np.ceil(tile_lo.max() / P))
    T_HI = int(np.ceil(tile_hi.max() / P))
    T = T_LO + T_HI

    # ---- host: edge partitioning via the assignment ----
    g_e = gtile_of[tgt]
    core = g_e // NT_PAD
    ntile = g_e - core * NT_PAD
    m_loc = m_of[tgt]
    sg_id = ntile // G
    half = half_e

    # group key: (core, supergroup, half, tile)
    gid = ((core * SG + sg_id) * 2 + half) * NT_PAD + ntile

    order = np.argsort(gid, kind="stable")
    src_s = src[order]
    et_s = et[order]
    ew_s = ew[order]
    half_s = half[order]
    core_s = core[order]
    sg_s = sg_id[order]
    ntile_s = ntile[order]
    m_s = m_loc[order]

    # slot of each edge in the flat per-core grid:
    # core-base + sg*slots_sg + half-block + (tile within sg)*Tx*P + pos
    gid_s = gid[order]
    starts = np.zeros(gid.max() + 2, dtype=np.int64)
    cnts = np.bincount(gid_s, minlength=gid.max() + 1)
    np.cumsum(cnts, out=starts[1:])
    pos = np.arange(E, dtype=np.int64) - starts[gid_s]

    slots_sg = G * T * P
    tile_in_sg = ntile_s - sg_s * G
    Tx = np.where(half_s == 0, T_LO, T_HI)
    block0 = np.where(half_s == 0, 0, G * T_LO * P)
    slot_in_sg = block0 + tile_in_sg * Tx * P + pos
    slot = (core_s * SG + sg_s) * slots_sg + slot_in_sg

    nslots = NCORES * SG * slots_sg
    idx_flat = np.zeros(nslots, dtype=np.int16)
    idx_flat[slot] = (src_s - half_s * SPLIT).astype(np.int16)

    # wrapped idx per (core, sg): lo block then hi block
    idx_rs = idx_flat.reshape(NCORES, SG, slots_sg)
    nlo = G * T_LO * P
    idx16 = np.empty((NCORES, SG, P, slots_sg // P * 8), dtype=np.int16)
    for c in range(NCORES):
        for s in range(SG):
            idx16[c, s, :, :nlo // 16] = _wrap16(idx_rs[c, s, :nlo])
            idx16[c, s, :, nlo // 16:] = _wrap16(idx_rs[c, s, nlo:])

    # ---- host: s4 selector, [core, sg, chunk, p, col] col = b*M + m ----
    bw16 = (bw.astype(np.float32))  # keep f32 for product, cast at store
    chunk_in_sg = slot_in_sg // P
    p_in_chunk = slot_in_sg % P
    s4 = np.zeros((NCORES, SG, G * T, P, B * M), dtype=ml_dtypes.bfloat16)
    cw = bw16[et_s] * ew_s[:, None]          # (E, B)
    for b in range(B):
        s4[core_s, sg_s, chunk_in_sg, p_in_chunk, b * M + m_s] = \
            cw[:, b].astype(ml_dtypes.bfloat16)
    s4 = s4.reshape(NCORES, SG, G * T, P, B * M).transpose(0, 1, 3, 2, 4)
    s4 = np.ascontiguousarray(s4).reshape(NCORES, SG, P, G * T * B * M)

    key = (N, D, NOUT, SG, T_LO, T_HI)
    if key not in _PROG_CACHE:
        _PROG_CACHE[key] = _build_program(*key)
    nc = _PROG_CACHE[key]

    in_maps = [
        dict(
            xlo=xlo,
            xhi=xhi,
            bases16=bases16,
            idx16=idx16[c],
            s4=s4[c],
        )
        for c in range(NCORES)
    ]
    res = run_bass_kernel_spmd(nc, in_maps, list(range(NCORES)), trace=TRACE)
    LAST_PROFILE = res
    out_dev = np.concatenate([res.results[c]["out"] for c in range(NCORES)],
                             axis=0)                    # [NCORES*NOUT, D]
    rows = gtile_of // NT_PAD * NOUT + (gtile_of % NT_PAD) * M + m_of
    return np.ascontiguousarray(out_dev[rows])


# revision 19
# speedup vs baseline: 1.3555x; 1.2456x over previous
"""BasesDecomposition GNN message passing on 8 Trainium2 NeuronCores.

Math (reference):
    seg  = edge_type * N + target
    h    = segment_sum(x[source] * ew, seg)        # (R, N, D)
    out  = einsum('rb,bio,rni->no', bw, bases, h)  # (N, D)

Restructured: fold the relation->basis projection and edge weight into a
per-edge selector row  s4[e, b*M + m] = bw[edge_type_e, b] * ew_e * [tgt==m]
so the accumulator shrinks to (B, M, D) per 32-node tile:
    pg[i, (b,m)] = sum_e xg[e, i] * s4[e, (b,m)]      (PE, edge tiles of 128)
    out[m, o]    = sum_b pg[:, (b,m)]^T @ bases[b]    (PE)

Sharding: nodes by target range across 8 cores (no collectives). Edges are
sorted by (supergroup of 8 tiles, src-half, tile) and packed into a uniform
slot grid (T_LO/T_HI 128-slot chunks per tile-half) so the SPMD program is
static. s4 is built entirely on the host (no DVE work per edge tile); x is
cast to bf16 on the host into two tables split at node 25000 so dma_gather
indices fit int16. Gather calls are 1024 idx each, spread round-robin over
4 SWDGE queues (queue q runs on Q7 core pair q -> ~4x descriptor-gen
throughput vs one queue).
"""

import numpy as np

import concourse.bass as bass
import concourse.mybir as mybir
import concourse.tile as tile
from concourse import bacc
from concourse.bass_utils import run_bass_kernel_spmd

NCORES = 8
P = 128          # edges per chunk (matmul contraction dim)
M = 32           # nodes per tile (selector block width; B*M = 128 = J)
B = 4            # num bases
G = 8            # tiles per supergroup (psum tiles live concurrently)
SPLIT = 22000    # x row split so gather indices fit int16 (tuned: T_LO=4, T_HI=5)
CALL = 1024      # idxs per dma_gather call (hard ucode cap)
NQ = 4           # SWDGE queues used round-robin
NT_PAD = 200     # node tiles per core (NCORES*NT_PAD*M = 51200 >= N slots)

TRACE = False
LAST_PROFILE = None

_PROG_CACHE = {}


def _build_program(N, D, NOUT, SG, T_LO, T_HI):
    fp = mybir.dt.float32
    bf = mybir.dt.bfloat16
    i16 = mybir.dt.int16
    T = T_LO + T_HI
    NLO = SPLIT
    NHI = N - SPLIT
    slots_sg = G * T * P              # slots per supergroup
    calls_lo = G * T_LO * P // CALL   # gather calls per supergroup (lo)
    calls_hi = G * T_HI * P // CALL
    assert G * T_LO * P % CALL == 0 and G * T_HI * P % CALL == 0

    nc = bacc.Bacc("TRN2", target_bir_lowering=False, debug=False,
                   num_devices=NCORES, num_swdge_queues=NQ)
    xlo_d = nc.dram_tensor("xlo", [NLO, D], bf, kind="ExternalInput").ap()
    xhi_d = nc.dram_tensor("xhi", [NHI, D], bf, kind="ExternalInput").ap()
    bases_d = nc.dram_tensor("bases16", [P, B * D], bf, kind="ExternalInput").ap()
    idx_d = nc.dram_tensor("idx16", [SG, P, slots_sg // P * 8], i16,
                           kind="ExternalInput").ap()
    s4_d = nc.dram_tensor("s4", [SG, P, G * T * B * M], bf,
                          kind="ExternalInput").ap()
    out_d = nc.dram_tensor("out", [NOUT, D], fp, kind="ExternalOutput").ap()

    with tile.TileContext(nc) as tc:
        with (
            tc.tile_pool(name="const", bufs=1) as constp,
            tc.tile_pool(name="s4p", bufs=3) as s4p,
            tc.tile_pool(name="xg", bufs=4) as xgp,
            tc.tile_pool(name="gsb", bufs=2) as gsbp,
            tc.tile_pool(name="osb", bufs=3) as osbp,
            tc.tile_pool(name="psg", bufs=4, space="PSUM") as psgp,
            tc.tile_pool(name="pso", bufs=2, space="PSUM") as psop,
        ):
            bases_sb = constp.tile([P, B * D], bf)
            nc.sync.dma_start(out=bases_sb[:], in_=bases_d[:])
            # preload ALL gather indices once (3.7 MB)
            idx_all = constp.tile([P, SG * (slots_sg // P * 8)], i16)
            icols = slots_sg // P * 8
            nc.sync.dma_start(
                out=idx_all[:].rearrange("p (s c) -> p s c", s=SG),
                in_=idx_d[:].rearrange("s p c -> p s c"),
            )

            call_no = 0
            for sg in range(SG):
                idxt = idx_all[:, sg * icols:(sg + 1) * icols]
                s4t = s4p.tile([P, G * T * B * M], bf, tag="s4")
                # split the s4 load into halves on alternating HWDGE rings
                h = G * T * B * M // 2
                nc.sync.dma_start(out=s4t[:, :h], in_=s4_d[sg][:, :h])
                nc.scalar.dma_start(out=s4t[:, h:], in_=s4_d[sg][:, h:])

                # gathers: lo block then hi block, 1024 idx per call
                xg_lo = xgp.tile([P, G * T_LO * D], bf, tag="xglo")
                xg_hi = xgp.tile([P, G * T_HI * D], bf, tag="xghi")
                i0 = sg * icols
                for k in range(calls_lo):
                    nc.gpsimd.dma_gather(
                        out_ap=xg_lo[:, k * 8 * D:(k * 8 + 8) * D].rearrange(
                            "p (t f) -> p t f", f=D),
                        in_ap=xlo_d[:],
                        idxs_ap=idx_all[:, i0 + k * 64:i0 + (k + 1) * 64],
                        num_idxs=CALL,
                        num_idxs_reg=CALL,
                        elem_size=D,
                        queue_num=call_no % NQ,
                    )
                    call_no += 1
                hi0 = i0 + calls_lo * 64
                for k in range(calls_hi):
                    nc.gpsimd.dma_gather(
                        out_ap=xg_hi[:, k * 8 * D:(k * 8 + 8) * D].rearrange(
                            "p (t f) -> p t f", f=D),
                        in_ap=xhi_d[:],
                        idxs_ap=idx_all[:, hi0 + k * 64:hi0 + (k + 1) * 64],
                        num_idxs=CALL,
                        num_idxs_reg=CALL,
                        elem_size=D,
                        queue_num=call_no % NQ,
                    )
                    call_no += 1

                # s4 chunk layout: lo chunks (G*T_LO) then hi chunks (G*T_HI),
                # in slot order; chunk c of tile q:
                #   lo: c = q*T_LO + t, hi: c = G*T_LO + q*T_HI + t
                pg = None
                for q in range(G):
                    qq = q % 4
                    if qq == 0:
                        pg = psgp.tile([P, 4 * B * M], fp)
                    for t in range(T_LO):
                        c = q * T_LO + t
                        nc.tensor.matmul(
                            out=pg[:, qq * B * M:(qq + 1) * B * M],
                            lhsT=xg_lo[:, c * D:(c + 1) * D],
                            rhs=s4t[:, c * B * M:(c + 1) * B * M],
                            start=(t == 0),
                            stop=False,
                        )
                    for t in range(T_HI):
                        c = q * T_HI + t
                        nc.tensor.matmul(
                            out=pg[:, qq * B * M:(qq + 1) * B * M],
                            lhsT=xg_hi[:, c * D:(c + 1) * D],
                            rhs=s4t[:, (G * T_LO + c) * B * M:
                                    (G * T_LO + c + 1) * B * M],
                            start=False,
                            stop=(t == T_HI - 1),
                        )
                    if qq == 3:
                        quad = q // 4
                        # transpose-copy psum [p,(q,b,m)] -> sbuf [p,(b,q,m)]
                        gsb = gsbp.tile([P, 4 * B * M], bf, tag="gsb")
                        nc.vector.tensor_copy(
                            out=gsb[:].rearrange("p (b q m) -> p b q m",
                                                 b=B, q=4),
                            in_=pg[:].rearrange("p (q b m) -> p b q m",
                                                q=4, b=B),
                        )
                        po = psop.tile([P, D], fp)
                        for b in range(B):
                            nc.tensor.matmul(
                                out=po[:],
                                lhsT=gsb[:, b * 4 * M:(b + 1) * 4 * M],
                                rhs=bases_sb[:, b * D:(b + 1) * D],
                                start=(b == 0),
                                stop=(b == B - 1),
                            )
                        m_lo = (sg * G + quad * 4) * M
                        osb = osbp.tile([P, D], fp)
                        nc.vector.tensor_copy(out=osb[:], in_=po[:])
                        nc.scalar.dma_start(out=out_d[m_lo:m_lo + 4 * M, :],
                                            in_=osb[:])
    nc.compile()
    return nc


def _wrap16(a):
    """Pack flat index array (n,) into dma_gather layout (128, n/16):
    index j lives at [j % 16, j // 16]; rows replicated to 128."""
    n = a.shape[0]
    w = a.reshape(n // 16, 16).T  # (16, n/16)
    return np.tile(w, (8, 1))


def kernel(x, source, target, edge_type, edge_weights, base_weights, bases):
    global LAST_PROFILE
    import ml_dtypes

    x = np.ascontiguousarray(np.asarray(x), dtype=np.float32)
    src = np.asarray(source).astype(np.int64)
    tgt = np.asarray(target).astype(np.int64)
    et = np.asarray(edge_type).astype(np.int64)
    ew = np.ascontiguousarray(np.asarray(edge_weights), dtype=np.float32)
    bw = np.ascontiguousarray(np.asarray(base_weights), dtype=np.float32)
    bs = np.ascontiguousarray(np.asarray(bases), dtype=np.float32)

    N, D = x.shape
    R, Bv = bw.shape
    E = src.shape[0]
    SG = NT_PAD // G
    NOUT = NT_PAD * M                 # device output rows per core

    # ---- host: bf16 tables (split so gather idx fits int16) ----
    xlo = x[:SPLIT].astype(ml_dtypes.bfloat16)
    xhi = x[SPLIT:].astype(ml_dtypes.bfloat16)
    bases16 = np.zeros((P, B * D), dtype=ml_dtypes.bfloat16)
    for b in range(B):
        bases16[:, b * D:(b + 1) * D] = bs[b].astype(ml_dtypes.bfloat16)

    # ---- host: balanced node -> (core, tile, m) assignment ----
    # Nodes can be placed in any output slot (we un-permute on the host), so
    # rank-match nodes onto tiles to equalize per-(tile, half) edge counts,
    # minimizing the uniform T padding of the static SPMD program.
    half_e = (src >= SPLIT).astype(np.int64)
    lo_deg = np.bincount(tgt[half_e == 0], minlength=N).astype(np.int64)
    hi_deg = np.bincount(tgt[half_e == 1], minlength=N).astype(np.int64)
    TILES = NCORES * NT_PAD
    node_order = np.argsort(-(lo_deg + hi_deg), kind="stable")
    tile_lo = np.zeros(TILES, dtype=np.int64)
    tile_hi = np.zeros(TILES, dtype=np.int64)
    tile_cnt = np.zeros(TILES, dtype=np.int64)
    gtile_of = np.empty(N, dtype=np.int64)
    m_of = np.empty(N, dtype=np.int64)
    for r0 in range(0, N, TILES):
        batch = node_order[r0:r0 + TILES]
        load = tile_lo * 5 + tile_hi * 4          # ~ lo/T_LO + hi/T_HI
        rank = np.argsort(load, kind="stable")
        chosen = rank[:len(batch)]
        gtile_of[batch] = chosen
        m_of[batch] = tile_cnt[chosen]
        tile_cnt[chosen] += 1
        tile_lo[chosen] += lo_deg[batch]
        tile_hi[chosen] += hi_deg[batch]
    assert tile_cnt.max() <= M

    T_LO = int(np.ceil(tile_lo.max() / P))
    T_HI = int(np.ceil(tile_hi.max() / P))
    T = T_LO + T_HI

    # ---- host: edge partitioning via the assignment ----
    g_e = gtile_of[tgt]
    core = g_e // NT_PAD
    ntile = g_e - core * NT_PAD
    m_loc = m_of[tgt]
    sg_id = ntile // G
    half = half_e

    # group key: (core, supergroup, half, tile)
    gid = ((core * SG + sg_id) * 2 + half) * NT_PAD + ntile

    order = np.argsort(gid, kind="stable")
    src_s = src[order]
    et_s = et[order]
    ew_s = ew[order]
    half_s = half[order]
    core_s = core[order]
    sg_s = sg_id[order]
    ntile_s = ntile[order]
    m_s = m_loc[order]

    # slot of each edge in the flat per-core grid:
    # core-base + sg*slots_sg + half-block + (tile within sg)*Tx*P + pos
    gid_s = gid[order]
    starts = np.zeros(gid.max() + 2, dtype=np.int64)
    cnts = np.bincount(gid_s, minlength=gid.max() + 1)
    np.cumsum(cnts, out=starts[1:])
    pos = np.arange(E, dtype=np.int64) - starts[gid_s]

    slots_sg = G * T * P
    tile_in_sg = ntile_s - sg_s * G
    Tx = np.where(half_s == 0, T_LO, T_HI)
    block0 = np.where(half_s == 0, 0, G * T_LO * P)
    slot_in_sg = block0 + tile_in_sg * Tx * P + pos
    slot = (core_s * SG + sg_s) * slots_sg + slot_in_sg

    nslots = NCORES * SG * slots_sg
    idx_flat = np.zeros(nslots, dtype=np.int16)
    idx_flat[slot] = (src_s - half_s * SPLIT).astype(np.int16)

    # wrapped idx per (core, sg): lo block then hi block
    idx_rs = idx_flat.reshape(NCORES, SG, slots_sg)
    nlo = G * T_LO * P
    idx16 = np.empty((NCORES, SG, P, slots_sg // P * 8), dtype=np.int16)
    for c in range(NCORES):
        for s in range(SG):
            idx16[c, s, :, :nlo // 16] = _wrap16(idx_rs[c, s, :nlo])
            idx16[c, s, :, nlo // 16:] = _wrap16(idx_rs[c, s, nlo:])

    # ---- host: s4 selector, [core, sg, chunk, p, col] col = b*M + m ----
    bw16 = (bw.astype(np.float32))  # keep f32 for product, cast at store
    chunk_in_sg = slot_in_sg // P
    p_in_chunk = slot_in_sg % P
    s4 = np.zeros((NCORES, SG, G * T, P, B * M), dtype=ml_dtypes.bfloat16)
    cw = bw16[et_s] * ew_s[:, None]          # (E, B)
    for b in range(B):
        s4[core_s, sg_s, chunk_in_sg, p_in_chunk, b * M + m_s] = \
            cw[:, b].astype(ml_dtypes.bfloat16)
    s4 = s4.reshape(NCORES, SG, G * T, P, B * M).transpose(0, 1, 3, 2, 4)
    s4 = np.ascontiguousarray(s4).reshape(NCORES, SG, P, G * T * B * M)

    key = (N, D, NOUT, SG, T_LO, T_HI)
    if key not in _PROG_CACHE:
        _PROG_CACHE[key] = _build_program(*key)
    nc = _PROG_CACHE[key]

    in_maps = [
        dict(
            xlo=xlo,
            xhi=xhi,
            bases16=bases16,
            idx16=idx16[c],
            s4=s4[c],
        )
        for c in range(NCORES)
    ]
    res = run_bass_kernel_spmd(nc, in_maps, list(range(NCORES)), trace=TRACE)
    LAST_PROFILE = res
    out_dev = np.concatenate([res.results[c]["out"] for c in range(NCORES)],
                             axis=0)                    # [NCORES*NOUT, D]
    rows = gtile_of // NT_PAD * NOUT + (gtile_of % NT_PAD) * M + m_of
    return np.ascontiguousarray(out_dev[rows])


# revision 22
# speedup vs baseline: 1.4159x; 1.0445x over previous
"""BasesDecomposition GNN message passing on 8 Trainium2 NeuronCores.

Math (reference):
    seg  = edge_type * N + target
    h    = segment_sum(x[source] * ew, seg)        # (R, N, D)
    out  = einsum('rb,bio,rni->no', bw, bases, h)  # (N, D)

Restructured: fold the relation->basis projection and edge weight into a
per-edge selector row  s4[e, b*M + m] = bw[edge_type_e, b] * ew_e * [tgt==m]
so the accumulator shrinks to (B, M, D) per 32-node tile:
    pg[i, (b,m)] = sum_e xg[e, i] * s4[e, (b,m)]      (PE, edge tiles of 128)
    out[m, o]    = sum_b pg[:, (b,m)]^T @ bases[b]    (PE)

Sharding: nodes by target range across 8 cores (no collectives). Edges are
sorted by (supergroup of 8 tiles, src-half, tile) and packed into a uniform
slot grid (T_LO/T_HI 128-slot chunks per tile-half) so the SPMD program is
static. s4 is built entirely on the host (no DVE work per edge tile); x is
cast to bf16 on the host into two tables split at node 25000 so dma_gather
indices fit int16. Gather calls are 1024 idx each, spread round-robin over
4 SWDGE queues (queue q runs on Q7 core pair q -> ~4x descriptor-gen
throughput vs one queue).
"""

import numpy as np

import concourse.bass as bass
import concourse.mybir as mybir
import concourse.tile as tile
from concourse import bacc
from concourse.bass_utils import run_bass_kernel_spmd

NCORES = 8
P = 128          # edges per chunk (matmul contraction dim)
M = 32           # nodes per tile (selector block width; B*M = 128 = J)
B = 4            # num bases
G = 8            # tiles per supergroup (psum tiles live concurrently)
SPLIT = 22000    # x row split so gather indices fit int16 (tuned: T_LO=4, T_HI=5)
CALL = 1024      # idxs per dma_gather call (hard ucode cap)
NQ = 4           # SWDGE queues used round-robin
NT_PAD = 200     # node tiles per core (NCORES*NT_PAD*M = 51200 >= N slots)

TRACE = False
LAST_PROFILE = None

_PROG_CACHE = {}


def _build_program(N, D, NOUT, SG, T_LO, T_HI):
    fp = mybir.dt.float32
    bf = mybir.dt.bfloat16
    i16 = mybir.dt.int16
    T = T_LO + T_HI
    NLO = SPLIT
    NHI = N - SPLIT
    slots_sg = G * T * P              # slots per supergroup
    calls_lo = G * T_LO * P // CALL   # gather calls per supergroup (lo)
    calls_hi = G * T_HI * P // CALL
    assert G * T_LO * P % CALL == 0 and G * T_HI * P % CALL == 0

    nc = bacc.Bacc("TRN2", target_bir_lowering=False, debug=False,
                   num_devices=NCORES, num_swdge_queues=NQ)
    xlo_d = nc.dram_tensor("xlo", [NLO, D], bf, kind="ExternalInput").ap()
    xhi_d = nc.dram_tensor("xhi", [NHI, D], bf, kind="ExternalInput").ap()
    bases_d = nc.dram_tensor("bases16", [P, B * D], bf, kind="ExternalInput").ap()
    idx_d = nc.dram_tensor("idx16", [SG, P, slots_sg // P * 8], i16,
                           kind="ExternalInput").ap()
    s4_d = nc.dram_tensor("s4", [SG, P, G * T * B * M], bf,
                          kind="ExternalInput").ap()
    out_d = nc.dram_tensor("out", [NOUT, D], fp, kind="ExternalOutput").ap()

    with tile.TileContext(nc) as tc:
        with (
            tc.tile_pool(name="const", bufs=1) as constp,
            tc.tile_pool(name="s4p", bufs=4) as s4p,
            tc.tile_pool(name="xg", bufs=3) as xgp,
            tc.tile_pool(name="gsb", bufs=2) as gsbp,
            tc.tile_pool(name="osb", bufs=3) as osbp,
            tc.tile_pool(name="psg", bufs=4, space="PSUM") as psgp,
            tc.tile_pool(name="pso", bufs=2, space="PSUM") as psop,
        ):
            bases_sb = constp.tile([P, B * D], bf)
            nc.sync.dma_start(out=bases_sb[:], in_=bases_d[:])
            # preload ALL gather indices (first sg alone so it starts fast)
            idx_all = constp.tile([P, SG * (slots_sg // P * 8)], i16)
            icols = slots_sg // P * 8
            nc.sync.dma_start(out=idx_all[:, :icols], in_=idx_d[0])
            nc.sync.dma_start(
                out=idx_all[:, icols:].rearrange("p (s c) -> p s c", s=SG - 1),
                in_=idx_d[1:].rearrange("s p c -> p s c"),
            )

            call_no = 0
            for sg in range(SG):
                s4t = s4p.tile([P, G * T * B * M], bf, tag="s4")
                # split the s4 load into quarters on alternating HWDGE rings
                qt = G * T * B * M // 4
                for j in range(4):
                    eng = nc.sync if j % 2 == 0 else nc.scalar
                    eng.dma_start(out=s4t[:, j * qt:(j + 1) * qt],
                                  in_=s4_d[sg][:, j * qt:(j + 1) * qt])

                # gathers: lo block then hi block, 1024 idx per call
                xg_lo = xgp.tile([P, G * T_LO * D], bf, tag="xglo")
                xg_hi = xgp.tile([P, G * T_HI * D], bf, tag="xghi")
                i0 = sg * icols
                for k in range(calls_lo):
                    nc.gpsimd.dma_gather(
                        out_ap=xg_lo[:, k * 8 * D:(k * 8 + 8) * D].rearrange(
                            "p (t f) -> p t f", f=D),
                        in_ap=xlo_d[:],
                        idxs_ap=idx_all[:, i0 + k * 64:i0 + (k + 1) * 64],
                        num_idxs=CALL,
                        num_idxs_reg=CALL,
                        elem_size=D,
                        queue_num=call_no % NQ,
                    )
                    call_no += 1
                hi0 = i0 + calls_lo * 64
                for k in range(calls_hi):
                    nc.gpsimd.dma_gather(
                        out_ap=xg_hi[:, k * 8 * D:(k * 8 + 8) * D].rearrange(
                            "p (t f) -> p t f", f=D),
                        in_ap=xhi_d[:],
                        idxs_ap=idx_all[:, hi0 + k * 64:hi0 + (k + 1) * 64],
                        num_idxs=CALL,
                        num_idxs_reg=CALL,
                        elem_size=D,
                        queue_num=call_no % NQ,
                    )
                    call_no += 1

                # s4 chunk layout: lo chunks (G*T_LO) then hi chunks (G*T_HI),
                # in slot order; chunk c of tile q:
                #   lo: c = q*T_LO + t, hi: c = G*T_LO + q*T_HI + t
                pg = None
                for q in range(G):
                    qq = q % 4
                    if qq == 0:
                        pg = psgp.tile([P, 4 * B * M], fp)
                    for t in range(T_LO):
                        c = q * T_LO + t
                        nc.tensor.matmul(
                            out=pg[:, qq * B * M:(qq + 1) * B * M],
                            lhsT=xg_lo[:, c * D:(c + 1) * D],
                            rhs=s4t[:, c * B * M:(c + 1) * B * M],
                            start=(t == 0),
                            stop=False,
                        )
                    for t in range(T_HI):
                        c = q * T_HI + t
                        nc.tensor.matmul(
                            out=pg[:, qq * B * M:(qq + 1) * B * M],
                            lhsT=xg_hi[:, c * D:(c + 1) * D],
                            rhs=s4t[:, (G * T_LO + c) * B * M:
                                    (G * T_LO + c + 1) * B * M],
                            start=False,
                            stop=(t == T_HI - 1),
                        )
                    if qq == 3:
                        quad = q // 4
                        # transpose-copy psum [p,(q,b,m)] -> sbuf [p,(b,q,m)]
                        gsb = gsbp.tile([P, 4 * B * M], bf, tag="gsb")
                        nc.vector.tensor_copy(
                            out=gsb[:].rearrange("p (b q m) -> p b q m",
                                                 b=B, q=4),
                            in_=pg[:].rearrange("p (q b m) -> p b q m",
                                                q=4, b=B),
                        )
                        po = psop.tile([P, D], fp)
                        for b in range(B):
                            nc.tensor.matmul(
                                out=po[:],
                                lhsT=gsb[:, b * 4 * M:(b + 1) * 4 * M],
                                rhs=bases_sb[:, b * D:(b + 1) * D],
                                start=(b == 0),
                                stop=(b == B - 1),
                            )
                        m_lo = (sg * G + quad * 4) * M
                        osb = osbp.tile([P, D], fp)
                        nc.vector.tensor_copy(out=osb[:], in_=po[:])
                        nc.scalar.dma_start(out=out_d[m_lo:m_lo + 4 * M, :],
                                            in_=osb[:])
    nc.compile()
    return nc


def _wrap16(a):
    """Pack flat index array (n,) into dma_gather layout (128, n/16):
    index j lives at [j % 16, j // 16]; rows replicated to 128."""
    n = a.shape[0]
    w = a.reshape(n // 16, 16).T  # (16, n/16)
    return np.tile(w, (8, 1))


def kernel(x, source, target, edge_type, edge_weights, base_weights, bases):
    global LAST_PROFILE
    import ml_dtypes

    x = np.ascontiguousarray(np.asarray(x), dtype=np.float32)
    src = np.asarray(source).astype(np.int64)
    tgt = np.asarray(target).astype(np.int64)
    et = np.asarray(edge_type).astype(np.int64)
    ew = np.ascontiguousarray(np.asarray(edge_weights), dtype=np.float32)
    bw = np.ascontiguousarray(np.asarray(base_weights), dtype=np.float32)
    bs = np.ascontiguousarray(np.asarray(bases), dtype=np.float32)

    N, D = x.shape
    R, Bv = bw.shape
    E = src.shape[0]
    SG = NT_PAD // G
    NOUT = NT_PAD * M                 # device output rows per core

    # ---- host: bf16 tables (split so gather idx fits int16) ----
    xlo = x[:SPLIT].astype(ml_dtypes.bfloat16)
    xhi = x[SPLIT:].astype(ml_dtypes.bfloat16)
    bases16 = np.zeros((P, B * D), dtype=ml_dtypes.bfloat16)
    for b in range(B):
        bases16[:, b * D:(b + 1) * D] = bs[b].astype(ml_dtypes.bfloat16)

    # ---- host: balanced node -> (core, tile, m) assignment ----
    # Nodes can be placed in any output slot (we un-permute on the host), so
    # rank-match nodes onto tiles to equalize per-(tile, half) edge counts,
    # minimizing the uniform T padding of the static SPMD program.
    half_e = (src >= SPLIT).astype(np.int64)
    lo_deg = np.bincount(tgt[half_e == 0], minlength=N).astype(np.int64)
    hi_deg = np.bincount(tgt[half_e == 1], minlength=N).astype(np.int64)
    TILES = NCORES * NT_PAD
    node_order = np.argsort(-(lo_deg + hi_deg), kind="stable")
    tile_lo = np.zeros(TILES, dtype=np.int64)
    tile_hi = np.zeros(TILES, dtype=np.int64)
    tile_cnt = np.zeros(TILES, dtype=np.int64)
    gtile_of = np.empty(N, dtype=np.int64)
    m_of = np.empty(N, dtype=np.int64)
    for r0 in range(0, N, TILES):
        batch = node_order[r0:r0 + TILES]
        load = tile_lo * 5 + tile_hi * 4          # ~ lo/T_LO + hi/T_HI
        rank = np.argsort(load, kind="stable")
        chosen = rank[:len(batch)]
        gtile_of[batch] = chosen
        m_of[batch] = tile_cnt[chosen]
        tile_cnt[chosen] += 1
        tile_lo[chosen] += lo_deg[batch]
        tile_hi[chosen] += hi_deg[batch]
    assert tile_cnt.max() <= M

    T_LO = int(np.ceil(tile_lo.max() / P))
    T_HI = int(np.ceil(tile_hi.max() / P))
    T = T_LO + T_HI

    # ---- host: edge partitioning via the assignment ----
    g_e = gtile_of[tgt]
    core = g_e // NT_PAD
    ntile = g_e - core * NT_PAD
    m_loc = m_of[tgt]
    sg_id = ntile // G
    half = half_e

    # group key: (core, supergroup, half, tile)
    gid = ((core * SG + sg_id) * 2 + half) * NT_PAD + ntile

    order = np.argsort(gid, kind="stable")
    src_s = src[order]
    et_s = et[order]
    ew_s = ew[order]
    half_s = half[order]
    core_s = core[order]
    sg_s = sg_id[order]
    ntile_s = ntile[order]
    m_s = m_loc[order]

    # slot of each edge in the flat per-core grid:
    # core-base + sg*slots_sg + half-block + (tile within sg)*Tx*P + pos
    gid_s = gid[order]
    starts = np.zeros(gid.max() + 2, dtype=np.int64)
    cnts = np.bincount(gid_s, minlength=gid.max() + 1)
    np.cumsum(cnts, out=starts[1:])
    pos = np.arange(E, dtype=np.int64) - starts[gid_s]

    slots_sg = G * T * P
    tile_in_sg = ntile_s - sg_s * G
    Tx = np.where(half_s == 0, T_LO, T_HI)
    block0 = np.where(half_s == 0, 0, G * T_LO * P)
    slot_in_sg = block0 + tile_in_sg * Tx * P + pos
    slot = (core_s * SG + sg_s) * slots_sg + slot_in_sg

    nslots = NCORES * SG * slots_sg
    idx_flat = np.zeros(nslots, dtype=np.int16)
    idx_flat[slot] = (src_s - half_s * SPLIT).astype(np.int16)

    # wrapped idx per (core, sg): lo block then hi block
    idx_rs = idx_flat.reshape(NCORES, SG, slots_sg)
    nlo = G * T_LO * P
    idx16 = np.empty((NCORES, SG, P, slots_sg // P * 8), dtype=np.int16)
    for c in range(NCORES):
        for s in range(SG):
            idx16[c, s, :, :nlo // 16] = _wrap16(idx_rs[c, s, :nlo])
            idx16[c, s, :, nlo // 16:] = _wrap16(idx_rs[c, s, nlo:])

    # ---- host: s4 selector, [core, sg, chunk, p, col] col = b*M + m ----
    bw16 = (bw.astype(np.float32))  # keep f32 for product, cast at store
    chunk_in_sg = slot_in_sg // P
    p_in_chunk = slot_in_sg % P
    s4 = np.zeros((NCORES, SG, G * T, P, B * M), dtype=ml_dtypes.bfloat16)
    cw = bw16[et_s] * ew_s[:, None]          # (E, B)
    for b in range(B):
        s4[core_s, sg_s, chunk_in_sg, p_in_chunk, b * M + m_s] = \
            cw[:, b].astype(ml_dtypes.bfloat16)
    s4 = s4.reshape(NCORES, SG, G * T, P, B * M).transpose(0, 1, 3, 2, 4)
    s4 = np.ascontiguousarray(s4).reshape(NCORES, SG, P, G * T * B * M)

    key = (N, D, NOUT, SG, T_LO, T_HI)
    if key not in _PROG_CACHE:
        _PROG_CACHE[key] = _build_program(*key)
    nc = _PROG_CACHE[key]

    in_maps = [
        dict(
            xlo=xlo,
            xhi=xhi,
            bases16=bases16,
            idx16=idx16[c],
            s4=s4[c],
        )
        for c in range(NCORES)
    ]
    res = run_bass_kernel_spmd(nc, in_maps, list(range(NCORES)), trace=TRACE)
    LAST_PROFILE = res
    out_dev = np.concatenate([res.results[c]["out"] for c in range(NCORES)],
                             axis=0)                    # [NCORES*NOUT, D]
    rows = gtile_of // NT_PAD * NOUT + (gtile_of % NT_PAD) * M + m_of
    return np.ascontiguousarray(out_dev[rows])
